# revision 9
# baseline (speedup 1.0000x reference)
"""CRF head kernel for Trainium2 (Bass/Tile), 8-core data-parallel.

Computes: out[b, t, :] = x[b, t, :] + transitions[argmax(x[b, t, :]), :]
for x of shape [128, 1024, 256] f32 and transitions [256, 256] f32.

Sharding: batch dim split across 8 NeuronCores (16 batches / core).
Per core: 16*1024 = 16384 rows, processed in megatiles of 1024 rows laid
out as [128 partitions, 8 groups, 256 tags] (each partition holds 8
consecutive rows -> contiguous 8KB DMA descriptors per partition).

The input data contains rows with exactly-tied maxima, and group-max
values that also occur in other groups of the same partition row; the
argmax must be first-occurrence WITHIN each group. Hence per-group
max_index (find_index8: consuming first-occurrence matcher) rather than
a one-shot is_equal one-hot or a flat 2048-wide search.

Engine balance (mandatory HBM traffic 33.5MB/core @ ~358GB/s ~= 94us is
the roofline):
  sync HWDGE   load 1MB x megatile
  DVE          reduce_max -> mx [128, 8]; 8x per-group max_index (first
               occurrence of mx within each 256-wide group); tiny add of
               group offsets -> scatter positions
  GpSimd       2x local_scatter: zeroes the one-hot half-tile and writes
               bf16 1.0 at each group's argmax position
  PE           per group: 2 transposes of one-hot halves -> PSUM bf16,
               then 3-matmul accumulation into psum_out:
                 ohT_lo.T @ trans_lo + ohT_hi.T @ trans_hi + I.T @ x_r
               (trans bf16: rel err ~2^-9; identity matmul in float32r
               runs at 1 cyc/row, folding "+x" into the PE accumulator)
  ACT          cast x -> float32r copy (fp32r operands must be produced
               as f32r; tf32-rounding x only perturbs the added VALUE by
               ~2.6e-3, the argmax path reads exact f32), plus batched
               PSUM->SBUF copies of transposed one-hot and result
  scalar HWDGE store 1MB out megatile (separate ring from loads)
"""

import sys

for _p in ("/opt/trn_rl_repo",):
    if _p not in sys.path:
        sys.path.append(_p)

import numpy as np

import concourse.bass as bass
import concourse.bacc as bacc
import concourse.mybir as mybir
import concourse.tile as tile
import concourse.bass_utils as bass_utils
from concourse.masks import make_identity

N_CORES = 8
B, T, TAGS = 128, 1024, 256
R = (B // N_CORES) * T          # rows per core = 16384
P = 128                         # SBUF partitions
G = 8                           # row-groups per megatile
ROWS_PER_MT = P * G             # 1024
M = R // ROWS_PER_MT            # 16 megatiles per core
GB = 4                          # groups per PSUM batch / scatter half

_CACHE = {}


def _build():
    nc = bacc.Bacc("TRN2", target_bir_lowering=False, debug=False)

    x = nc.dram_tensor("x", [R, TAGS], mybir.dt.float32, kind="ExternalInput")
    t = nc.dram_tensor("t", [TAGS, TAGS], mybir.dt.float32, kind="ExternalInput")
    y = nc.dram_tensor("y", [R, TAGS], mybir.dt.float32, kind="ExternalOutput")

    # megatile m, partition p holds rows m*1024 + p*8 .. +7 (contiguous)
    xv = x.ap().rearrange("(m p g) d -> m p (g d)", p=P, g=G)
    yv = y.ap().rearrange("(m p g) d -> m p (g d)", p=P, g=G)

    with tile.TileContext(nc) as tc:
        with (
            tc.tile_pool(name="cp", bufs=1) as cp,
            tc.tile_pool(name="xp", bufs=3) as xp,
            tc.tile_pool(name="xrp", bufs=2) as xrp,
            tc.tile_pool(name="ohp", bufs=4) as ohp,
            tc.tile_pool(name="ohtp", bufs=3) as ohtp,
            tc.tile_pool(name="outp", bufs=3) as outp,
            tc.tile_pool(name="mxp", bufs=4) as mxp,
            tc.psum_pool(name="ptp", bufs=2) as ptp,
            tc.psum_pool(name="pop", bufs=3) as pop,
        ):
            # ---- one-time constants -------------------------------------
            tr_f = cp.tile([P, 2 * TAGS], mybir.dt.float32, tag="trf",
                           name="tr_f")
            nc.sync.dma_start(
                out=tr_f[:].rearrange("p (h n) -> p h n", h=2),
                in_=t.ap().rearrange("(h k) n -> k h n", h=2))
            tr_b = cp.tile([P, 2 * TAGS], mybir.dt.bfloat16, tag="trb",
                           name="tr_b")
            nc.vector.tensor_copy(out=tr_b[:], in_=tr_f[:])

            id_b = cp.tile([P, P], mybir.dt.bfloat16, tag="idb", name="id_b")
            make_identity(nc, id_b[:])
            id_f = cp.tile([P, P], mybir.dt.float32, tag="idf", name="id_f")
            make_identity(nc, id_f[:])
            id_r = cp.tile([P, P], mybir.dt.float32r, tag="idr", name="id_r")
            nc.vector.tensor_copy(out=id_r[:], in_=id_f[:])

            ones4 = cp.tile([P, GB], mybir.dt.bfloat16, tag="ones",
                            name="ones4")
            nc.gpsimd.memset(ones4[:], 1.0)
            # scatter offsets per group within its half: [0,256,512,768]x2
            offs = cp.tile([P, G], mybir.dt.uint16, tag="offs", name="offs")
            nc.gpsimd.iota(offs[:], pattern=[[0, 2], [TAGS, GB]], base=0,
                           channel_multiplier=0)

            for m in range(M):
                x_t = xp.tile([P, G * TAGS], mybir.dt.float32, tag="x",
                              name=f"x_{m}")
                nc.sync.dma_start(out=x_t[:], in_=xv[m])
                x3 = x_t[:].rearrange("p (g d) -> p g d", d=TAGS)

                mx = mxp.tile([P, G], mybir.dt.float32, tag="mx",
                              name=f"mx_{m}")
                nc.vector.tensor_reduce(out=mx[:], in_=x3,
                                        axis=mybir.AxisListType.X,
                                        op=mybir.AluOpType.max)

                # per-group first-occurrence argmax via find_index8
                mi = mxp.tile([P, G, 8], mybir.dt.uint16, tag="mi",
                              name=f"mi_{m}")
                for g in range(G):
                    mx_g = mx[:, g:g + 1]
                    mx_b8 = bass.AP(mx_g.tensor, mx_g.offset,
                                    [list(mx_g.ap[0]), [0, 8]])
                    nc.vector.max_index(out=mi[:, g, :], in_max=mx_b8,
                                        in_values=x3[:, g, :])
                pos = mxp.tile([P, G], mybir.dt.uint16, tag="pos",
                               name=f"pos_{m}")
                nc.vector.tensor_tensor(out=pos[:], in0=mi[:, :, 0],
                                        in1=offs[:],
                                        op=mybir.AluOpType.add)

                x_r = xrp.tile([P, G * TAGS], mybir.dt.float32r, tag="xr",
                               name=f"xr_{m}")
                nc.scalar.copy(out=x_r[:], in_=x_t[:])
                xr3 = x_r[:].rearrange("p (g d) -> p g d", d=TAGS)

                out_t = outp.tile([P, G * TAGS], mybir.dt.float32, tag="out",
                                  name=f"out_{m}")

                for b in range(G // GB):
                    # one-hot half-tile: zeroed + bf16 ones scattered at
                    # each group's argmax position
                    oh = ohp.tile([P, GB * TAGS], mybir.dt.bfloat16,
                                  tag="oh", name=f"oh_{m}_{b}")
                    nc.gpsimd.local_scatter(
                        out_ap=oh[:],
                        data_ap=ones4[:],
                        idxs_ap=pos[:, b * GB:(b + 1) * GB].bitcast(
                            mybir.dt.int16),
                        channels=P,
                        num_elems=GB * TAGS,
                        num_idxs=GB,
                    )
                    oh3 = oh[:].rearrange("p (g d) -> p g d", d=TAGS)

                    pt = ptp.tile([P, GB, 2, P], mybir.dt.bfloat16, tag="pt",
                                  name=f"pt_{m}_{b}")
                    for gl in range(GB):
                        nc.tensor.transpose(pt[:, gl, 0, :], oh3[:, gl, 0:P],
                                            id_b[:])
                        nc.tensor.transpose(pt[:, gl, 1, :],
                                            oh3[:, gl, P:TAGS], id_b[:])
                    ohT = ohtp.tile([P, GB, 2, P], mybir.dt.bfloat16,
                                    tag="ohT", name=f"ohT_{m}_{b}")
                    nc.scalar.copy(out=ohT[:], in_=pt[:])

                    po = pop.tile([P, GB, TAGS], mybir.dt.float32, tag="po",
                                  name=f"po_{m}_{b}")
                    for gl in range(GB):
                        g = b * GB + gl
                        nc.tensor.matmul(po[:, gl, :], lhsT=ohT[:, gl, 0, :],
                                         rhs=tr_b[:, 0:TAGS],
                                         start=True, stop=False)
                        nc.tensor.matmul(po[:, gl, :], lhsT=ohT[:, gl, 1, :],
                                         rhs=tr_b[:, TAGS:2 * TAGS],
                                         start=False, stop=False)
                        nc.tensor.matmul(po[:, gl, :], lhsT=id_r[:],
                                         rhs=xr3[:, g, :],
                                         start=False, stop=True)

                    w = GB * TAGS
                    nc.scalar.copy(out=out_t[:, b * w:(b + 1) * w],
                                   in_=po[:].rearrange("p g d -> p (g d)"))

                nc.scalar.dma_start(out=yv[m], in_=out_t[:])

    nc.compile()
    return nc


def get_nc():
    if "nc" not in _CACHE:
        _CACHE["nc"] = _build()
    return _CACHE["nc"]


def kernel(launch_matrix, transitions):
    launch = np.ascontiguousarray(np.asarray(launch_matrix, dtype=np.float32))
    trans = np.ascontiguousarray(np.asarray(transitions, dtype=np.float32))
    assert launch.shape == (B, T, TAGS), launch.shape
    assert trans.shape == (TAGS, TAGS), trans.shape

    nc = get_nc()
    shards = launch.reshape(N_CORES, R, TAGS)
    in_maps = [{"x": shards[c], "t": trans} for c in range(N_CORES)]
    res = bass_utils.run_bass_kernel_spmd(nc, in_maps,
                                          core_ids=list(range(N_CORES)))
    _CACHE["last_results"] = res
    out = np.concatenate([res.results[c]["y"] for c in range(N_CORES)], axis=0)
    return out.reshape(B, T, TAGS)
